# revision 1
# baseline (speedup 1.0000x reference)
"""Trainium2 Bass kernel for nn_AttenLayer (ragged-sequence attention pooling).

Math (per batch b, with length L_b):
    proj   = tanh(nn_outs @ W^T + b)           # (S, A)
    scores = proj @ context                     # (S,)
    atten  = masked_softmax(scores, L_b)        # (S,), zeros beyond L_b
    out    = atten @ nn_outs                    # (H,)

Sharding: pure data-parallel over batch; 8 batches per core on 8 cores.

Per-core plan (all matmuls bf16, f32 PSUM accumulation):
  - nn_outs (host-cast to bf16) is loaded twice per batch:
      natural [s128, h512] tiles  (rhs of phase-3, k=s)
      xbar-DMA-transposed [h128, s2048] tiles (rhs of phase-1, k=h)
  - phase 1: projT[a128, s512] psum = sum_h  W^T[h,a] @ xT[h,s]
    tanh+bias on ACT -> projT sbuf (bf16)
  - scores[1, s512] psum = sum_a context[a,1] @ projT[a,s]
  - batches processed in 2 waves of 4; per-wave masked softmax on a
    [4, 2048] tile (iota/len compare builds the mask; exp on ACT with
    fused accumulate for the denominator)
  - atten [4,128] chunks transposed on PE -> attT [128, (j,b)] bf16
  - phase 3: out[1, h512] psum = sum_s attT[s,1] @ nat[s,h]
"""

import sys

for _p in ("/opt/trn_rl_repo",):
    if _p not in sys.path:
        sys.path.insert(0, _p)

import numpy as np
import ml_dtypes

import concourse.bass as bass
from concourse import bacc
import concourse.mybir as mybir
import concourse.tile as tile
from concourse.masks import make_identity

B, S, H, A = 64, 2048, 512, 512
NCORES = 8
BPC = B // NCORES          # batches per core
WAVE_SIZES = [4, 2, 2]
WB = 4                     # max wave size (ctx zero-pad layout width)
WAVE_SPANS = []
_o = 0
for _wsz in WAVE_SIZES:
    WAVE_SPANS.append((_o, _wsz))
    _o += _wsz

SJ = S // 512              # 4  s-chunks of 512 (phase-1 N)
SK = S // 128              # 16 s-chunks of 128 (phase-3 K)
AC = A // 128              # 4  a-chunks
HC = H // 128              # 4  h-chunks

F32 = mybir.dt.float32
BF16 = mybir.dt.bfloat16



def build_nc(repeat: int = 1) -> bass.Bass:
    nc = bacc.Bacc()

    x_bf = nc.declare_dram_parameter("x_bf", [BPC, S, H], BF16, isOutput=False)
    xt_d = nc.declare_dram_parameter("xt_d", [BPC, H, S], BF16, isOutput=False)
    # W^T pre-chunked on host: wt[p, c*A + a] = proj_w[a, 128c + p]
    wt_d = nc.declare_dram_parameter("wt", [128, HC * A], BF16, isOutput=False)
    ctx_d = nc.declare_dram_parameter("ctx", [128, AC * WB * WB], BF16, isOutput=False)
    pb_d = nc.declare_dram_parameter("pb", [128, AC], F32, isOutput=False)
    mask_d = nc.declare_dram_parameter("mask", [BPC, S], BF16, isOutput=False)
    out_d = nc.declare_dram_parameter("out", [BPC, H], F32, isOutput=True)

    with tile.TileContext(nc) as tc:
        with (
            tc.tile_pool(name="const", bufs=1) as const_pool,
            tc.tile_pool(name="nat", bufs=6) as nat_pool,
            tc.tile_pool(name="xt", bufs=8) as xt_pool,
            tc.tile_pool(name="projT", bufs=6) as proj_pool,
            tc.tile_pool(name="smx", bufs=2) as smx_pool,
            tc.tile_pool(name="attT", bufs=8) as attT_pool,
            tc.tile_pool(name="osb", bufs=8) as os_pool,
            tc.tile_pool(name="p1ps", bufs=2, space="PSUM") as p1_psum,
            tc.tile_pool(name="scps", bufs=4, space="PSUM") as sc_psum,
            tc.tile_pool(name="atps", bufs=1, space="PSUM") as at_psum,
            tc.tile_pool(name="ops", bufs=1, space="PSUM") as out_psum,
        ):
            # ---- constants (DMAs deferred to first use; see trace_batch) ----
            wt_sb = const_pool.tile([128, HC * A], BF16, tag="wt")
            ctx_sb = const_pool.tile([128, AC * WB * WB], BF16, tag="ctx")
            pb_sb = const_pool.tile([128, AC], F32, tag="pb")
            mask_w = {}
            ident = const_pool.tile([128, 128], F32, tag="ident")
            make_identity(nc, ident[:])

            nat = {}    # b -> natural tile, layout [p, (n, h)]
            scpss = {}  # w -> list of 4 scores psum tiles [wb, 512]
            attT = {}   # (w, g) -> [128, 4*wb] bf16, col = wb*jj + bw

            def trace_batch(w, b0, wb, bwi):
                b = b0 + bwi
                xts = []
                for hc in range(HC):
                    xt = xt_pool.tile([128, S], BF16, tag="xt")
                    xts.append(xt)
                if b == 0:
                    # j-chunked + wt interleaved so the first matmuls start
                    # after ~256KB of DMA instead of the whole preamble
                    for j in range(SJ):
                        jsl = slice(j * 512, (j + 1) * 512)
                        for hc in range(HC):
                            nc.sync.dma_start(
                                xts[hc][:, jsl],
                                xt_d[b, hc * 128 : (hc + 1) * 128, jsl],
                            )
                            if j == 0:
                                nc.sync.dma_start(
                                    wt_sb[:, hc * A : (hc + 1) * A],
                                    wt_d[:, hc * A : (hc + 1) * A],
                                )
                        if j == 0:
                            nc.sync.dma_start(pb_sb[:], pb_d[:])
                            nc.sync.dma_start(ctx_sb[:], ctx_d[:])
                else:
                    for hc in range(HC):
                        nc.sync.dma_start(
                            xts[hc][:], xt_d[b, hc * 128 : (hc + 1) * 128, :]
                        )
                natb = nat_pool.tile([128, SK * 512], BF16, tag="nat")
                nat[b] = natb
                nc.sync.dma_start(
                    natb[:].rearrange("p (n h) -> p n h", n=SK),
                    x_bf[b].rearrange("(n p) h -> p n h", p=128),
                )
                for j in range(SJ):
                    for a in range(AC):
                        ps = p1_psum.tile([128, 512], F32, tag="p1")
                        for hc in range(HC):
                            nc.tensor.matmul(
                                ps[:],
                                wt_sb[:, hc * A + a * 128 : hc * A + (a + 1) * 128],
                                xts[hc][:, j * 512 : (j + 1) * 512],
                                start=(hc == 0),
                                stop=(hc == HC - 1),
                            )
                        pt = proj_pool.tile([128, 512], BF16, tag="projT")
                        nc.scalar.activation(
                            pt[:],
                            ps[:],
                            mybir.ActivationFunctionType.Tanh,
                            bias=pb_sb[:, a : a + 1],
                        )
                        # ctx col bw is context's a-chunk, others zero, so only
                        # row bw of the wave's scores psum accumulates batch b.
                        nc.tensor.matmul(
                            scpss[w][j][:],
                            ctx_sb[:, (a * WB + bwi) * WB : (a * WB + bwi) * WB + wb],
                            pt[:],
                            start=(bwi == 0 and a == 0),
                            stop=(bwi == wb - 1 and a == AC - 1),
                        )

            def finish_wave(w, b0, wb):
                if w not in mask_w:
                    mw = const_pool.tile([wb, S], BF16, tag=f"mask{w}")
                    nc.sync.dma_start(mw[:], mask_d[b0 : b0 + wb, :])
                    mask_w[w] = mw
                # masked softmax: scm = scores + additive mask, chunkwise
                scm = smx_pool.tile([wb, S], F32, tag="scm")
                pmax = smx_pool.tile([wb, SJ], F32, tag="pmax")
                for j in range(SJ):
                    sl = slice(j * 512, (j + 1) * 512)
                    nc.vector.tensor_tensor(
                        out=scm[:, sl], in0=scpss[w][j][:],
                        in1=mask_w[w][:, sl], op=mybir.AluOpType.add,
                    )
                    nc.vector.reduce_max(
                        pmax[:, j : j + 1], scm[:, sl], axis=mybir.AxisListType.X
                    )
                mx = smx_pool.tile([wb, 1], F32, tag="mx")
                nc.vector.reduce_max(
                    mx[:], pmax[:], axis=mybir.AxisListType.X, negate=True
                )
                ex = smx_pool.tile([wb, S], BF16, tag="ex")
                rs = smx_pool.tile([wb, 1], F32, tag="rs")
                nc.scalar.activation(
                    ex[:],
                    scm[:],
                    mybir.ActivationFunctionType.Exp,
                    bias=mx[:],
                    accum_out=rs[:],
                )
                rv = smx_pool.tile([wb, 1], F32, tag="rv")
                nc.vector.reciprocal(rv[:], rs[:])
                at = smx_pool.tile([wb, S], F32, tag="at")
                nc.scalar.activation(
                    at[:], ex[:], mybir.ActivationFunctionType.Copy, scale=rv[:]
                )
                # transpose atten chunks [wb, 128] -> [128, wb] on PE
                for g in range(SK // 4):
                    aps = at_psum.tile([128, 4 * wb], F32, tag="atps")
                    for jj in range(4):
                        j = 4 * g + jj
                        nc.tensor.transpose(
                            aps[:, jj * wb : (jj + 1) * wb],
                            at[:, j * 128 : (j + 1) * 128],
                            ident[:wb, :wb],
                        )
                    att_sb = attT_pool.tile([128, 4 * wb], BF16, tag="attT")
                    nc.vector.tensor_copy(att_sb[:], aps[:])
                    attT[(w, g)] = att_sb
                # phase 3: out[b] = sum_s atten[s] * x[s, :]
                for bwi in range(wb):
                    b = b0 + bwi
                    ops = out_psum.tile([1, 512], F32, tag="ops")
                    for j in range(SK):
                        col = (j % 4) * wb + bwi
                        nc.tensor.matmul(
                            ops[:],
                            attT[(w, j // 4)][:, col : col + 1],
                            nat[b][:, j * 512 : (j + 1) * 512],
                            start=(j == 0),
                            stop=(j == SK - 1),
                        )
                    os_b = os_pool.tile([1, H], F32, tag="os")
                    nc.any.tensor_copy(os_b[:], ops[:])
                    nc.sync.dma_start(out_d[b : b + 1, :], os_b[:])

            for _rep in range(repeat):
                nat.clear(); scpss.clear(); attT.clear()
                for w, (b0, wb) in enumerate(WAVE_SPANS):
                    scps_j = []
                    for _j in range(SJ):
                        scps = sc_psum.tile([wb, 512], F32, tag="scps")
                        scps_j.append(scps)
                    scpss[w] = scps_j
                    for bwi in range(wb):
                        trace_batch(w, b0, wb, bwi)
                        if bwi == 0 and w > 0:
                            pb0, pwb = WAVE_SPANS[w - 1]
                            finish_wave(w - 1, pb0, pwb)
                b0, wb = WAVE_SPANS[-1]
                finish_wave(len(WAVE_SPANS) - 1, b0, wb)

    nc.finalize()
    return nc


_NC = None


def get_nc() -> bass.Bass:
    global _NC
    if _NC is None:
        _NC = build_nc(repeat=1)
    return _NC


def make_in_maps(nn_outs, batch_lens, context, proj_w, proj_b):
    """Host-side shard prep. Returns list of per-core input dicts."""
    x_bf = np.asarray(nn_outs, dtype=np.float32).astype(ml_dtypes.bfloat16)
    xt_host = np.ascontiguousarray(x_bf.transpose(0, 2, 1))  # [B, H, S]
    wt = np.ascontiguousarray(np.asarray(proj_w, np.float32).T)  # [H, A]
    # wt_sb[p, c*A + a] = wt[128c + p, a]
    wt_host = np.ascontiguousarray(
        wt.reshape(HC, 128, A).transpose(1, 0, 2).reshape(128, HC * A)
    ).astype(ml_dtypes.bfloat16)
    ctx_c = np.asarray(context, np.float32).reshape(AC, 128)
    ctx_host = np.zeros((128, AC, WB, WB), np.float32)
    for a in range(AC):
        for bw in range(WB):
            ctx_host[:, a, bw, bw] = ctx_c[a]
    ctx_host = np.ascontiguousarray(
        ctx_host.reshape(128, AC * WB * WB)
    ).astype(ml_dtypes.bfloat16)
    pb_host = np.ascontiguousarray(
        np.asarray(proj_b, np.float32).reshape(AC, 128).T
    )
    lens_full = np.asarray(batch_lens).reshape(B)
    nj_full = np.minimum((lens_full + 511) // 512, SJ).astype(np.int64)
    # balance total chunk count across cores: sort desc, greedy to lightest core
    order = np.argsort(-nj_full, kind="stable")
    core_of = np.empty(B, np.int64)
    loads = [0.0 + 1e-9 * c for c in range(NCORES)]
    counts = [0] * NCORES
    for b in order:
        c = min(
            (c for c in range(NCORES) if counts[c] < BPC),
            key=lambda c: loads[c],
        )
        core_of[b] = c
        loads[c] += nj_full[b]
        counts[c] += 1
    perm = np.argsort(core_of, kind="stable")  # batches grouped by core
    lens = lens_full[perm].reshape(NCORES, BPC)
    nj = nj_full[perm].astype(np.int32).reshape(NCORES, 1, BPC)
    iota = np.arange(S)[None, :]
    mask_add = np.where(iota < lens.reshape(-1, 1), 0.0, -30000.0).astype(
        ml_dtypes.bfloat16
    ).reshape(NCORES, BPC, S)
    in_maps = []
    for c in range(NCORES):
        in_maps.append(
            {
                "x_bf": np.ascontiguousarray(x_bf[perm[c * BPC : (c + 1) * BPC]]),
                "xt_d": np.ascontiguousarray(xt_host[perm[c * BPC : (c + 1) * BPC]]),
                "wt": wt_host,
                "ctx": ctx_host,
                "pb": pb_host,
                "mask": np.ascontiguousarray(mask_add[c]),
            }
        )
    return in_maps, perm


def run(nn_outs, batch_lens, context, proj_w, proj_b, trace=False, **trace_kw):
    from concourse.bass_utils import run_bass_kernel_spmd

    nc = get_nc()
    in_maps, perm = make_in_maps(nn_outs, batch_lens, context, proj_w, proj_b)
    res = run_bass_kernel_spmd(
        nc, in_maps, list(range(NCORES)), trace=trace, **trace_kw
    )
    out_p = np.concatenate(
        [res.results[c]["out"] for c in range(NCORES)], axis=0
    )
    out = np.empty_like(out_p)
    out[perm] = out_p
    return out.astype(np.float32), res


def kernel(nn_outs, batch_lens, context, proj_w, proj_b):
    out, _ = run(nn_outs, batch_lens, context, proj_w, proj_b, trace=False)
    return out

